# revision 20
# baseline (speedup 1.0000x reference)
"""TRN2 Bass kernel for the NTK-track Conv1d problem.

Reference computation (per batch element b, all fp32):
    xv = relu(x[...,0]); x0 = relu(x[...,1]); dx = x[...,2] * (x[...,1] >= 0)
    s = sqrt(|alpha|)  (per-tap scale, K=9)
    x_out  = conv1d(xv, weight*s)/sqrt(C) + bias*sqrt(|beta|)
    x0_out = conv1d(x0, w0*s)/sqrt(C)     + b0*sqrt(|beta|)
    dx_out = (conv1d(dx, w0*s) + conv1d(x0, w*s))/sqrt(C) + b*sqrt(|beta|)
    out = stack([x_out, x0_out, dx_out], -1)

Shapes: x (8, 256, 8192, 3); weight/w0/w (256, 256, 9); pad=4 (same conv).

Strategy: data-parallel over batch (8 cores, 1 batch element each).
The elementwise track prep (relu / heaviside mask) and all scalar weight
folds (sqrt|alpha|, 1/sqrt C, sqrt|beta|·bias) happen on the host; tracks
and weights ship to the device as bf16 (the PE streams bf16 at the same
1 column/cycle as float32r, but DMA bytes halve; conv error ~1.5e-3).
Per core, conv1d(track, W) is 9*2 shifted 128x128x512 matmuls accumulated
in PSUM (contraction over C and tap k).

Schedule (cost-model-verified: PE has zero idle gaps between the first
real matmul and the last):
  * startup: DMAs issued in exact consumption order (tile-0 tracks as
    separate per-track tiles, first weight piece split by tap) so the
    first conv matmul starts ~4.2us in, right at the serial-DMA bound;
  * a burst of 128-col scratch warmup matmuls keeps the PE busy from
    ~1.4us so the p-state ramp completes during the DMA prefix;
  * group-major conv order (x for both oc halves, then x0, then dx)
    matches the weight arrival order with ~2us slack per group;
  * each PSUM bank is evicted eagerly (DVE bias-add) and the final dx
    group runs as four 128-col quarters on alternating PSUM banks so
    the last eviction + output DMA trail only ~3.8us after the last
    matmul.
"""

import math

import numpy as np

B, C, O, T, K = 8, 256, 256, 8192, 9
PAD = 4
P = 128  # partitions
TT = 512  # time-tile (matmul free dim)
NT = T // TT  # 16 time tiles
CCH = C // P  # 2 contraction chunks
OCH = O // P  # 2 output-partition chunks
HALO = TT + 2 * PAD  # 520 input columns per tile
TP = T + 2 * PAD  # host-padded track length
NCORES = 8


def _split_excess_waits(nc) -> int:
    """Move excess per-instruction semaphore waits onto standalone
    EventSemaphore carrier instructions.

    The walrus build in this environment rejects any instruction carrying
    more than ONE sync wait at codegen ("Too many sync wait commands");
    Tile's sem assignment freely emits several. Walk the finished BIR and
    hoist overflow waits onto fresh same-engine EventSemaphore instructions
    placed immediately before the over-budget instruction.
    """
    import concourse.mybir as mybir

    n_carriers = 0
    for f in nc.m.functions:
        for blk in f.blocks:
            insts = list(blk.instructions)
            new_insts = []
            dirty = False
            for inst in insts:
                si = inst.sync_info
                waits = list(si.on_wait) if si is not None and si.on_wait else []
                if len(waits) > 1:
                    overflow, keep = waits[:-1], waits[-1:]
                    for w in overflow:
                        ev = mybir.InstEventSemaphore(
                            name=f"{inst.name}_waitc{n_carriers}",
                            engine=inst.engine,
                        )
                        ev.sync_info = mybir.SyncInfo(on_wait=[w], on_update=[])
                        nc.register_instruction(ev, overwrite=True)
                        new_insts.append(ev)
                        n_carriers += 1
                    upd = list(si.on_update) if si.on_update else []
                    inst.sync_info = mybir.SyncInfo(on_wait=keep, on_update=upd)
                    dirty = True
                new_insts.append(inst)
            if dirty:
                blk.instructions = new_insts
    return n_carriers


def _build_nc(n_warm: int = 28):
    import concourse.bass as bass
    import concourse.mybir as mybir
    from concourse.tile import TileContext

    f32 = mybir.dt.float32
    bf16 = mybir.dt.bfloat16

    nc = bass.Bass()
    # tracks, host-prepped (relu/mask applied, zero-padded to T+8), bf16
    xd = nc.declare_dram_parameter("xd", [C, 3 * TP], bf16, isOutput=False)
    # weights in [p_c, oc, cc, k, p_o] layout so per-(oc,cc) DMA pieces are
    # contiguous and arrive in matmul consumption order
    w1 = nc.declare_dram_parameter("w1", [P, OCH * CCH * K * P], bf16, isOutput=False)
    w2 = nc.declare_dram_parameter("w2", [P, OCH * CCH * K * P], bf16, isOutput=False)
    w3 = nc.declare_dram_parameter("w3", [P, OCH * CCH * K * P], bf16, isOutput=False)
    bs = nc.declare_dram_parameter("bs", [P, OCH * 3], f32, isOutput=False)
    yd = nc.declare_dram_parameter("yd", [C, T * 3], f32, isOutput=True)

    with TileContext(nc) as tc:
        with (
            tc.tile_pool(name="wpool", bufs=1) as wpool,
            tc.tile_pool(name="trks", bufs=6) as trks,
            tc.tile_pool(name="opool", bufs=4) as opool,
            tc.tile_pool(name="psum", bufs=2, space="PSUM") as psp,
            tc.tile_pool(name="qps", bufs=1, space="PSUM") as qps,
        ):
            # --- PE warmup: scratch matmuls with no DMA dependency keep the
            # PE busy from t~0 so the p-state ramp finishes during the DMA
            # startup prefix. Reads a memset tile, writes a dead PSUM bank.
            warm = wpool.tile([P, P], bf16)
            nc.vector.memset(warm[:], 0.0)
            wps_t = qps.tile([P, P], f32, tag="psq0")
            for _ in range(n_warm):
                nc.tensor.matmul(
                    wps_t[:], warm[:], warm[:],
                    start=True, stop=True,
                )

            # Persistent weights / biases
            w1s = wpool.tile([P, OCH, CCH, K, P], bf16)
            w2s = wpool.tile([P, OCH, CCH, K, P], bf16)
            w3s = wpool.tile([P, OCH, CCH, K, P], bf16)
            bss = wpool.tile([P, OCH, 3], f32)

            def wview(w):
                return w[:].rearrange(
                    "p (o c k q) -> p o c k q", o=OCH, c=CCH, k=K
                )

            xv = xd[:].rearrange("c (s t) -> c s t", s=3)

            def load_slab(tt):
                """Fused per-cc slab: one DMA, accessor returns track views.
                Tile deps are whole-tile, so consumers wait for the full
                3-track DMA — fine in steady state."""
                t0 = tt * TT
                tiles = []
                for cc in range(CCH):
                    trk = trks.tile([P, 3, HALO], bf16, tag="trk")
                    nc.sync.dma_start(
                        trk[:],
                        xv[cc * P : (cc + 1) * P, :, t0 : t0 + HALO],
                    )
                    tiles.append(lambda s, _t=trk: _t[:, s])
                return tiles

            # --- startup: DMAs issued in exact consumption order so the
            # first real matmul starts as early as the serial DMA device
            # allows and never starves afterwards. Tile 0's tracks are
            # SEPARATE tiles per track (whole-tile deps: a fused slab would
            # stall the first matmuls on all three track DMAs).
            t0_tiles = [[None] * 3 for _ in range(CCH)]

            def load_track0(cc, s):
                trk = trks.tile([P, HALO], bf16, tag=f"trk0_{cc}_{s}")
                nc.sync.dma_start(trk[:], xv[cc * P : (cc + 1) * P, s, 0:HALO])
                t0_tiles[cc][s] = trk

            slab_cache = {
                0: [lambda s, _c=cc: t0_tiles[_c][s][:] for cc in range(CCH)]
            }
            load_track0(0, 0)
            nc.sync.dma_start(w1s[:, 0, 0, 0:3], wview(w1)[:, 0, 0, 0:3])  # oc0 cc0 k0-2
            nc.sync.dma_start(w1s[:, 0, 0, 3:K], wview(w1)[:, 0, 0, 3:K])  # oc0 cc0 k3-8
            load_track0(1, 0)
            nc.sync.dma_start(w1s[:, 0, 1], wview(w1)[:, 0, 1])  # oc0 cc1
            nc.sync.dma_start(bss[:], bs[:].rearrange("p (o s) -> p o s", o=OCH))
            nc.sync.dma_start(w1s[:, 1], wview(w1)[:, 1])        # oc1
            load_track0(0, 1)
            load_track0(1, 1)
            nc.sync.dma_start(w2s[:, 0], wview(w2)[:, 0])
            nc.sync.dma_start(w2s[:, 1], wview(w2)[:, 1])
            load_track0(0, 2)
            load_track0(1, 2)
            nc.sync.dma_start(w3s[:, 0], wview(w3)[:, 0])
            nc.sync.dma_start(w3s[:, 1], wview(w3)[:, 1])
            slab_cache[1] = load_slab(1)
            slab_cache[2] = load_slab(2)

            for tt in range(NT):
                t0 = tt * TT
                tracks = slab_cache.pop(tt) if tt in slab_cache else load_slab(tt)
                # prefetch ahead (pool bufs=4 gives 4 generations in flight)
                if tt + 3 < NT and tt + 3 not in slab_cache:
                    slab_cache[tt + 3] = load_slab(tt + 3)
                def conv_group(ps, oc, wsets, lo=0, hi=TT):
                    """wsets: list of (weight_tile, track_idx)."""
                    n = len(wsets) * CCH * K
                    i = 0
                    for ws, s in wsets:
                        for cc in range(CCH):
                            for k in range(K):
                                nc.tensor.matmul(
                                    ps[:], ws[:, oc, cc, k],
                                    tracks[cc](s)[:, lo + k : hi + k],
                                    start=(i == 0), stop=(i == n - 1),
                                )
                                i += 1

                # group-major (track-major) order: all-oc x convs, then x0,
                # then dx. This matches the weight DMA arrival order at
                # startup (w1 before w2 before w3) with ~2us of slack, and
                # evicts each PSUM bank eagerly.
                ots = []
                for _oc in range(OCH):
                    ot = opool.tile([P, TT, 3], f32, tag="ot", name=f"ot{_oc}")
                    ots.append(ot)
                for oc in range(OCH):
                    ps_x = psp.tile([P, TT], f32, tag="psx")
                    conv_group(ps_x, oc, [(w1s, 0)])
                    nc.vector.tensor_scalar_add(ots[oc][:, :, 0], ps_x[:], bss[:, oc, 0:1])
                for oc in range(OCH):
                    ps_x0 = psp.tile([P, TT], f32, tag="psx0")
                    conv_group(ps_x0, oc, [(w2s, 1)])
                    nc.vector.tensor_scalar_add(ots[oc][:, :, 1], ps_x0[:], bss[:, oc, 1:2])
                for oc in range(OCH):
                    if not (tt == NT - 1 and oc == OCH - 1):
                        ps_dx = psp.tile([P, TT], f32, tag="psdx")
                        conv_group(ps_dx, oc, [(w2s, 2), (w3s, 1)])
                        nc.vector.tensor_scalar_add(ots[oc][:, :, 2], ps_dx[:], bss[:, oc, 2:3])
                        nc.sync.dma_start(
                            yd[oc * P : (oc + 1) * P, 3 * t0 : 3 * (t0 + TT)],
                            ots[oc][:].rearrange("p t s -> p (t s)"),
                        )
                    else:
                        # final group: run dx as four 128-col quarters on
                        # alternating PSUM tiles so each quarter's eviction
                        # + output DMA overlap the next quarter's matmuls,
                        # shrinking the kernel tail
                        bounds = [0, 128, 256, 384, TT]
                        for h in range(len(bounds) - 1):
                            lo, hi = bounds[h], bounds[h + 1]
                            ps_q = qps.tile([P, hi - lo], f32, tag=f"psq{h % 2}",
                                            name=f"psq_{h}")
                            conv_group(ps_q, oc, [(w2s, 2), (w3s, 1)],
                                       lo=lo, hi=hi)
                            nc.vector.tensor_scalar_add(
                                ots[oc][:, lo:hi, 2],
                                ps_q[:],
                                bss[:, oc, 2:3],
                            )
                            nc.sync.dma_start(
                                yd[oc * P : (oc + 1) * P,
                                   3 * (t0 + lo) : 3 * (t0 + hi)],
                                ots[oc][:, lo:hi].rearrange("p t s -> p (t s)"),
                            )

    _split_excess_waits(nc)
    return nc


_CACHE: dict = {}


def _prep_weights(weight, w0, w, alpha):
    """(O, C, K) fp32 -> bf16 lhsT layout [p_c, oc, cc, k, p_o] flat."""
    import ml_dtypes

    s = np.sqrt(np.abs(np.asarray(alpha, np.float32)))  # (1,1,K)
    inv_sqrt_c = np.float32(1.0 / math.sqrt(C))
    out = []
    for wt in (weight, w0, w):
        wt = np.asarray(wt, np.float32) * s * inv_sqrt_c  # (O, C, K)
        wt = wt.reshape(OCH, P, CCH, P, K).transpose(3, 0, 2, 4, 1)
        out.append(
            np.ascontiguousarray(wt)
            .reshape(P, OCH * CCH * K * P)
            .astype(ml_dtypes.bfloat16)
        )
    return out


def _prep_tracks(x):
    """(B, C, T, 3) fp32 -> per-core [C, 3, T+2*PAD] bf16 with relu/mask
    applied and zero edge padding baked in."""
    import ml_dtypes

    x = np.asarray(x, np.float32)
    xv = np.maximum(x[..., 0], 0.0)
    x0 = x[..., 1]
    dx = x[..., 2] * (x0 >= 0).astype(np.float32)
    x0 = np.maximum(x0, 0.0)
    tr = np.zeros((B, C, 3, TP), np.float32)
    tr[:, :, 0, PAD : PAD + T] = xv
    tr[:, :, 1, PAD : PAD + T] = x0
    tr[:, :, 2, PAD : PAD + T] = dx
    return tr.astype(ml_dtypes.bfloat16)


def kernel(x, weight, w0, w, alpha, bias, b0, b, beta):
    from concourse.bass_utils import run_bass_kernel_spmd

    w1_np, w2_np, w3_np = _prep_weights(weight, w0, w, alpha)
    tracks = _prep_tracks(x)
    sb = np.float32(math.sqrt(abs(float(np.asarray(beta)))))
    biases = np.stack(
        [np.asarray(bias, np.float32) * sb,
         np.asarray(b0, np.float32) * sb,
         np.asarray(b, np.float32) * sb],
        axis=-1,
    )  # (O, 3) in track order [x, x0, dx]
    bs_np = np.ascontiguousarray(biases.reshape(OCH, P, 3).transpose(1, 0, 2)).reshape(
        P, OCH * 3
    )

    if "nc" not in _CACHE:
        _CACHE["nc"] = _build_nc()
    nc = _CACHE["nc"]

    in_maps = []
    for c in range(NCORES):
        in_maps.append(
            {
                "xd": np.ascontiguousarray(tracks[c].reshape(C, 3 * TP)),
                "w1": w1_np,
                "w2": w2_np,
                "w3": w3_np,
                "bs": bs_np,
            }
        )
    res = run_bass_kernel_spmd(nc, in_maps, list(range(NCORES)))
    out = np.empty((B, C, T, 3), np.float32)
    for c in range(NCORES):
        out[c] = res.results[c]["yd"].reshape(C, T, 3)
    return out
